# revision 22
# baseline (speedup 1.0000x reference)
"""Trainium2 Bass kernel for chunked (= full, non-causal) cross-attention.

  out = softmax((query Wq^T)(context Wk^T)^T / sqrt(d_head)) (context Wv^T) Wo^T

Shapes: query [2, 2048, 1024], context [2, 4096, 1024], W* [1024, 1024],
16 heads x 64 dims.

Distribution: tensor-parallel over heads.  Core c owns heads {2c, 2c+1}
(128 of the 1024 head dims) for both batches: it holds 128-row slices of
Wq/Wk/Wv and the matching 128-column slice of Wo and computes a
full-shape partial output.

I/O is sharded to minimize host<->device traffic (the axon tunnel is
~40MB/s, so replicated activations dominated wall-clock): each core
receives only a 128-row slice of the transposed activations (1/8 of
qT/cT), device-side AllGathers reassemble the full activations in local
DRAM before compute, and a device-side ReduceScatter performs the TP
all-reduce of the partial outputs so each core returns just its 1/8
slice of the final output.  Host traffic: ~30MB in + ~17MB out, vs
~330MB for the replicated/All-host variant.

On-device layout notes:
  * Activations are fed TRANSPOSED (qT/cT: [B, D, T]) and in bf16 so every
    DMA is contiguous and matmul contraction dims land on partitions.
  * Scores are computed transposed (S^T [k, q]) so softmax's sum over k is
    the AV matmul's contraction; the denominator Z rides along as a fused
    ones-column in the AV stationary operand (M = 64+1).
  * exp runs on the scalar (ACT) engine straight out of PSUM with the
    1/sqrt(64) folded into the activation's free scale; no max-subtraction
    is needed (scores are ~N(0,1); exp stays far below fp32/bf16 limits).
"""

import os
from contextlib import ExitStack

import numpy as np
import ml_dtypes

# Persistent XLA compilation cache: the axon PJRT path re-jits a fresh
# closure every run_bass_kernel_spmd call, which re-runs the BIR->NEFF
# compile (~0.5s).  The persistent cache keys on the HLO fingerprint, so
# warm calls (and future processes) skip it.
try:
    import jax

    jax.config.update("jax_compilation_cache_dir", "/tmp/jax_bass_cache")
    jax.config.update("jax_persistent_cache_min_entry_size_bytes", -1)
    jax.config.update("jax_persistent_cache_min_compile_time_secs", 0)
except Exception:
    pass

import concourse.bass as bass
import concourse.tile as tile
from concourse import bacc, mybir
from concourse.bass_utils import run_bass_kernel_spmd
from concourse.masks import make_identity

B = 2
TQ = 2048
TC = 4096
D = 1024
H = 16
DH = 64
NCORES = 8
E = 128          # head dims owned per core (2 heads x 64)
CT = D // 128    # contraction tiles over d_model
KT = TC // 128   # 128-wide key tiles
QC = TQ // 512   # 512-wide query chunks
KC = TC // 512   # 512-wide key chunks (projection moving dim)
G = 1024         # AllGather chunk granularity (tokens)
CCH = TC // G    # context AG chunks per batch
QCH = TQ // G    # query AG chunks per batch

BF16 = mybir.dt.bfloat16
F32 = mybir.dt.float32

_CACHE = {}
DEBUG = bool(int(os.environ.get("KBG_DEBUG", "0")))


RG = [list(range(NCORES))]


def _build_kernel():
    """Build + compile the per-core Bass module (identical on all cores)."""
    nc = bacc.Bacc(
        "TRN2", target_bir_lowering=False, debug=False, num_devices=NCORES
    )

    # sharded I/O, chunk-major so each AllGather output is contiguous:
    # core c holds rows [128c, 128c+128) of qT/cT per (batch, G-chunk)
    qT_s = nc.dram_tensor("qT_s", [B, QCH, 128, G], BF16, kind="ExternalInput").ap()
    cT_s = nc.dram_tensor("cT_s", [B, CCH, 128, G], BF16, kind="ExternalInput").ap()
    wq = nc.dram_tensor("wq", [D, E], BF16, kind="ExternalInput").ap()
    wk = nc.dram_tensor("wk", [D, E], BF16, kind="ExternalInput").ap()
    wv = nc.dram_tensor("wv", [D, E], BF16, kind="ExternalInput").ap()
    wo = nc.dram_tensor("wo", [64, 2, D], BF16, kind="ExternalInput").ap()
    # reduce-scattered output: core c keeps rows [128c, 128c+128) of
    # out^T per (batch, 512-wide q-chunk)
    out_s = nc.dram_tensor(
        "out_s", [B, QC, 128, 512], BF16, kind="ExternalOutput"
    ).ap()

    # internal DRAM: collective bounce buffers (collectives cannot touch
    # External I/O tensors directly) + gathered activations + TP partials
    qsb = nc.dram_tensor("qsb", [B, QCH, 128, G], BF16, kind="Internal").ap()
    csb = nc.dram_tensor("csb", [B, CCH, 128, G], BF16, kind="Internal").ap()
    qT = nc.dram_tensor("qT_full", [B, QCH, D, G], BF16, kind="Internal").ap()
    cT = nc.dram_tensor("cT_full", [B, CCH, D, G], BF16, kind="Internal").ap()
    out_t = nc.dram_tensor("out_part", [B, QC, D, 512], F32, kind="Internal").ap()
    rs_out = nc.dram_tensor("rs_out", [B, QC, 128, 512], F32, kind="Internal").ap()
    ext = {"out_s": out_s, "rs_out": rs_out}

    dbg = {}
    if DEBUG:
        for name, shape, dt in [
            ("d_qts", [128, TQ], BF16),
            ("d_kts", [128, TC], BF16),
            ("d_vsb", [128, KT, 2, 65], BF16),
            ("d_pt", [128, 2, 512], BF16),
            ("d_rz", [1, 2, 512], F32),
            ("d_rzb", [64, 2, 512], F32),
            ("d_att", [64, 2, 512], BF16),
        ]:
            dbg[name] = nc.dram_tensor(name, shape, dt, kind="ExternalOutput").ap()

    with tile.TileContext(nc) as tc:
        with ExitStack() as ctx:
            # Bounce the input shards to Internal DRAM, then AllGather the
            # full activations chunk by chunk.  Chunk order matches the
            # projection stream's consumption order (c0,q0,c1..c3,q1 per
            # batch), so only the first ~2 chunks are exposed; the rest of
            # the CC stream runs underneath compute.
            nc.sync.dma_start(csb, cT_s)
            nc.sync.dma_start(qsb, qT_s)

            def ag(src, dst):
                nc.gpsimd.collective_compute(
                    "AllGather", mybir.AluOpType.bypass, replica_groups=RG,
                    ins=[src.opt()], outs=[dst.opt()],
                )

            for b in range(B):
                ag(csb[b, 0], cT[b, 0])
                ag(qsb[b, 0], qT[b, 0])
                for cc in range(1, CCH):
                    ag(csb[b, cc], cT[b, cc])
                for qq in range(1, QCH):
                    ag(qsb[b, qq], qT[b, qq])

            _body(ctx, tc, qT, cT, wq, wk, wv, wo, out_t, ext, dbg)

    nc.compile()
    return nc


def _body(ctx, tc, qT, cT, wq, wk, wv, wo, out_t, ext, dbg=None):
    nc = tc.nc
    out_s, rs_out = ext["out_s"], ext["rs_out"]

    const = ctx.enter_context(tc.tile_pool(name="const", bufs=1))
    xq_pool = ctx.enter_context(tc.tile_pool(name="xq", bufs=3))
    xc_pool = ctx.enter_context(tc.tile_pool(name="xc", bufs=4))
    qts_pool = ctx.enter_context(tc.tile_pool(name="qts", bufs=2))
    kts_pool = ctx.enter_context(tc.tile_pool(name="kts", bufs=2))
    vts_pool = ctx.enter_context(tc.tile_pool(name="vts", bufs=1))
    v_pool = ctx.enter_context(tc.tile_pool(name="vsb", bufs=2))
    pt_pool = ctx.enter_context(tc.tile_pool(name="pt", bufs=10))
    avs_pool = ctx.enter_context(tc.tile_pool(name="avs", bufs=2))
    rz_pool = ctx.enter_context(tc.tile_pool(name="rz", bufs=2))
    rzb_pool = ctx.enter_context(tc.tile_pool(name="rzb", bufs=2))
    att_pool = ctx.enter_context(tc.tile_pool(name="att", bufs=2))
    vstage_pool = ctx.enter_context(tc.tile_pool(name="vstage", bufs=4))
    osb_pool = ctx.enter_context(tc.tile_pool(name="osb", bufs=4))
    cast_pool = ctx.enter_context(tc.tile_pool(name="cast", bufs=2))
    dram_pool = ctx.enter_context(tc.tile_pool(name="dram", bufs=2, space="DRAM"))

    sc_psum = ctx.enter_context(tc.tile_pool(name="sc_ps", bufs=2, space="PSUM"))
    av_psum = ctx.enter_context(tc.tile_pool(name="av_ps", bufs=2, space="PSUM"))
    # proj + Wo chains share one double-buffered pool; both are paced
    # one-instruction-at-a-time into the attention stream, so the FIFO
    # slot order can't serialize whole phases against each other.
    misc_psum = ctx.enter_context(tc.tile_pool(name="mi_ps", bufs=2, space="PSUM"))

    # --- constants -----------------------------------------------------
    ident = const.tile([128, 128], BF16)
    make_identity(nc, ident)
    wq_sb = const.tile([128, CT, E], BF16)
    wk_sb = const.tile([128, CT, E], BF16)
    wv_sb = const.tile([128, CT, E], BF16)
    for w_hbm, w_sb in ((wq, wq_sb), (wk, wk_sb), (wv, wv_sb)):
        nc.sync.dma_start(w_sb, w_hbm.rearrange("(ct p) e -> p ct e", p=128))
    wo_sb = const.tile([64, 2, D], BF16)
    nc.sync.dma_start(wo_sb, wo)

    def proj_gen(b, out):
        """Project one batch.  Yields after each PE matmul so the caller
        can pace this work into the attention stream of the previous
        batch (keeps the PE busy but never bursty enough to starve the
        exp pipeline)."""
        # Input chunks live in small ring buffers: slot WAR is at chunk
        # granularity, so the next batch's loads start as soon as this
        # batch's corresponding chains finish (instead of waiting for the
        # whole activation buffer to be released).
        xc_chunks = [None] * KC
        xq_chunks = [None] * QC
        n5 = G // 512  # 512-chunks per AG chunk

        def load_xc(c):
            t = xc_pool.tile([128, CT, 512], BF16, tag="xc")
            src = cT[b, c // n5].rearrange("(ct p) t -> p ct t", p=128)
            nc.sync.dma_start(t, src[:, :, bass.ts(c % n5, 512)])
            xc_chunks[c] = t

        def load_xq(c):
            t = xq_pool.tile([128, CT, 512], BF16, tag="xq")
            src = qT[b, c // n5].rearrange("(ct p) t -> p ct t", p=128)
            nc.sync.dma_start(t, src[:, :, bass.ts(c % n5, 512)])
            xq_chunks[c] = t

        kTs = kts_pool.tile([128, TC], BF16, tag="kts")
        qTs = qts_pool.tile([128, TQ], BF16, tag="qts")
        vTs = vts_pool.tile([128, TC], BF16, tag="vts")
        v_sb = v_pool.tile([128, KT, 2, 65], BF16, tag="vsb")
        nc.vector.memset(v_sb[:, :, :, 64:65], 1.0)
        out.update(kTs=kTs, qTs=qTs, v_sb=v_sb)

        def chain(w_sb, src, dst, c):
            ps = misc_psum.tile([128, 512], F32, tag="mi")
            for ct in range(CT):
                nc.tensor.matmul(
                    ps, w_sb[:, ct, :], src[:, ct, :],
                    start=(ct == 0), stop=(ct == CT - 1),
                )
                yield
            nc.vector.tensor_copy(dst[:, bass.ts(c, 512)], ps)

        def v_transpose(kt):
            # PE transpose: DMA-transpose would force xbar-mode transitions
            # against the copy DMAs sharing the HWDGE queues, which
            # serialize the whole DMA stream (measured as multi-us exp
            # stalls whenever transposes were in flight).
            tp = misc_psum.tile([128, 2, 64], BF16, tag="mi")
            nc.tensor.transpose(tp, vTs[:, bass.ts(kt, 128)], ident)
            nc.vector.tensor_copy(v_sb[:, kt, :, 0:64], tp)
            yield

        # Emission order is a schedule: the PE executes in order, so each
        # chunk must be emitted before the attention iterations that read
        # it.  kt-iteration 4c reads K_c (scores) and V_c (AV), so those
        # chains are emitted V-then-K per chunk; Q_c is only needed when
        # q-chunk c starts, so Q1..Q3 trail at the end.
        load_xc(0)
        load_xq(0)
        load_xc(1)
        yield from chain(wk_sb, xc_chunks[0], kTs, 0)
        yield from chain(wq_sb, xq_chunks[0], qTs, 0)
        load_xc(2)
        yield from chain(wv_sb, xc_chunks[0], vTs, 0)
        for kt in range(4):
            yield from v_transpose(kt)
        for c in range(1, KC):
            if c + 2 < KC:
                load_xc(c + 2)
            yield from chain(wk_sb, xc_chunks[c], kTs, c)
            yield from chain(wv_sb, xc_chunks[c], vTs, c)
            for kt in range(4 * c, 4 * c + 4):
                yield from v_transpose(kt)
        for c in range(1, QC):
            load_xq(c)
            yield from chain(wq_sb, xq_chunks[c], qTs, c)

    def wo_gen(b, qc, att):
        """Output projection for one q-chunk; paced like proj_gen.  Ends
        with the TP reduce-scatter of this q-chunk's partials and the bf16
        downcast of this core's slice, so the collective stream trails the
        compute by ~one q-chunk and only the last one is exposed."""
        for mt in range(D // 128):
            wops = misc_psum.tile([128, 512], F32, tag="mi")
            nc.tensor.matmul(
                wops, wo_sb[:, 0, bass.ts(mt, 128)], att[:, 0, :],
                start=True, stop=False,
            )
            yield
            nc.tensor.matmul(
                wops, wo_sb[:, 1, bass.ts(mt, 128)], att[:, 1, :],
                start=False, stop=True,
            )
            yield
            osb = osb_pool.tile([128, 512], F32, tag="osb")
            nc.vector.tensor_copy(osb, wops)
            nc.sync.dma_start(out_t[b, qc, bass.ts(mt, 128), :], osb)
            yield
        nc.gpsimd.collective_compute(
            "ReduceScatter", mybir.AluOpType.add, replica_groups=RG,
            ins=[out_t[b, qc].opt()], outs=[rs_out[b, qc].opt()],
        )
        yield
        t32 = cast_pool.tile([128, 512], F32, tag="c32")
        nc.sync.dma_start(t32, rs_out[b, qc])
        t16 = cast_pool.tile([128, 512], BF16, tag="c16")
        nc.vector.tensor_copy(t16, t32)
        nc.sync.dma_start(out_s[b, qc], t16)
        yield

    def drive(gens, n):
        done = 0
        while gens and done < n:
            try:
                next(gens[0])
                done += 1
            except StopIteration:
                gens.pop(0)

    proj_pending = []
    wo_pending = []

    # Batch 0: emit loads + chunk-0 projections up front; the rest is
    # paced into the attention stream below (emission position == the
    # PE's execution position, so pacing IS the schedule).
    tensors = [{}, {}]
    proj_pending.append(proj_gen(0, tensors[0]))
    drive(proj_pending, 29)

    for b in range(B):
        kTs, qTs, v_sb = (tensors[b][k] for k in ("kTs", "qTs", "v_sb"))
        if b + 1 < B:
            proj_pending.append(proj_gen(b + 1, tensors[b + 1]))

        for qc in range(QC):
            av0 = av_psum.tile([65, 512], F32, tag="av")
            av1 = av_psum.tile([65, 512], F32, tag="av")
            for kt in range(KT):
                # paced interleave first: producers must be emitted ahead
                # of the iterations that consume them.
                if b == 0 and qc == 0:
                    drive(proj_pending, 5)
                else:
                    drive(proj_pending, 2)
                if kt % 2 == 0:
                    drive(wo_pending, 1)
                sc = sc_psum.tile([128, 2, 512], F32, tag="sc")
                # scores^T [k, q] for the two heads, row-tiled (d=64 each)
                nc.tensor.matmul(
                    sc[:, 0, :], kTs[0:64, bass.ts(kt, 128)],
                    qTs[0:64, bass.ts(qc, 512)], start=True, stop=True,
                )
                nc.tensor.matmul(
                    sc[:, 1, :], kTs[64:128, bass.ts(kt, 128)],
                    qTs[64:128, bass.ts(qc, 512)], start=True, stop=True,
                )
                pt = pt_pool.tile([128, 2, 512], BF16, tag="pt")
                nc.scalar.activation(
                    pt, sc, mybir.ActivationFunctionType.Exp, scale=0.125,
                )
                # AV (+ ones row -> Z at output row 64), accumulate over kt
                nc.tensor.matmul(
                    av0, v_sb[:, kt, 0, :], pt[:, 0, :],
                    start=(kt == 0), stop=(kt == KT - 1),
                )
                nc.tensor.matmul(
                    av1, v_sb[:, kt, 1, :], pt[:, 1, :],
                    start=(kt == 0), stop=(kt == KT - 1),
                )

            # --- stage AV+Z out of PSUM immediately (frees the banks so
            # the next q-chunk starts without draining the pipeline; the
            # slow normalize chain runs on SBUF copies, off the critical
            # path) ----------------------------------------------------
            avs = avs_pool.tile([65, 2, 512], F32, tag="avs")
            nc.vector.tensor_copy(avs[:, 0, :], av0)
            nc.vector.tensor_copy(avs[:, 1, :], av1)

            # --- softmax normalization --------------------------------
            rz = rz_pool.tile([128, 2, 512], F32, tag="rz")
            nc.vector.reciprocal(rz[64:65, :, :], avs[64:65, :, :])
            # Broadcast 1/Z along partitions via a DRAM bounce (engines
            # can't move data across partitions; DMA with a 0-step
            # partition dim from DRAM can).
            rzd = dram_pool.tile([2, 512], F32, tag="rzd")
            nc.sync.dma_start(rzd[0:1, :], rz[64:65, 0, :])
            nc.sync.dma_start(rzd[1:2, :], rz[64:65, 1, :])
            rzb = rzb_pool.tile([64, 2, 512], F32, tag="rzb")
            for j in range(2):
                s = rzd[j : j + 1, :]
                src = bass.AP(
                    tensor=s.tensor, offset=s.offset,
                    ap=[[0, 64]] + [list(d) for d in s.ap[1:]],
                )
                nc.gpsimd.dma_start(rzb[:, j, :], src)
            att = att_pool.tile([64, 2, 512], BF16, tag="att")
            nc.vector.tensor_mul(att[:, 0, :], avs[0:64, 0, :], rzb[:, 0, :])
            nc.vector.tensor_mul(att[:, 1, :], avs[0:64, 1, :], rzb[:, 1, :])

            wo_pending.append(wo_gen(b, qc, att))

    # drain whatever interleaved work remains
    drive(proj_pending, 1 << 30)
    drive(wo_pending, 1 << 30)


def _prep_inputs(query, context, Wq, Wk, Wv, Wo):
    """Host-side sharding: bf16 casts, transposes, per-core weight slices."""
    bf16 = ml_dtypes.bfloat16

    def t_bf16(x):
        """Cast then blocked-transpose [B,T,D] -> [B,D,T]; the straight
        strided astype reads one cache line per 2-byte element."""
        b, t, d = x.shape
        x16 = x.astype(bf16)
        out = np.empty((b, d, t), bf16)
        xt = x16.transpose(0, 2, 1)
        for t0 in range(0, t, 256):
            out[:, :, t0 : t0 + 256] = xt[:, :, t0 : t0 + 256]
        return out

    qT = t_bf16(query)
    cT = t_bf16(context)
    in_maps = []
    for c in range(NCORES):
        sl = slice(E * c, E * (c + 1))
        rs = slice(128 * c, 128 * (c + 1))
        wo_slice = np.ascontiguousarray(Wo[:, sl].T)          # [128 e, 1024 m]
        wo_dev = np.ascontiguousarray(
            wo_slice.reshape(2, 64, D).transpose(1, 0, 2)      # [64, 2, 1024]
        ).astype(bf16)
        in_maps.append({
            "qT_s": np.ascontiguousarray(
                qT[:, rs, :].reshape(B, 128, QCH, G).transpose(0, 2, 1, 3)
            ),
            "cT_s": np.ascontiguousarray(
                cT[:, rs, :].reshape(B, 128, CCH, G).transpose(0, 2, 1, 3)
            ),
            "wq": np.ascontiguousarray(Wq[sl, :].T).astype(bf16),
            "wk": np.ascontiguousarray(Wk[sl, :].T).astype(bf16),
            "wv": np.ascontiguousarray(Wv[sl, :].T).astype(bf16),
            "wo": wo_dev,
        })
    return in_maps


def run(query, context, Wq, Wk, Wv, Wo, trace=False):
    """Run on 8 cores; returns (full output [B, TQ, D] fp32, BassKernelResults)."""
    if "nc" not in _CACHE:
        _CACHE["nc"] = _build_kernel()
    nc = _CACHE["nc"]
    in_maps = _prep_inputs(query, context, Wq, Wk, Wv, Wo)
    try:
        res = run_bass_kernel_spmd(
            nc, in_maps, core_ids=list(range(NCORES)), trace=trace,
        )
    except Exception:
        # transient device/tunnel states (e.g. a prior process killed
        # mid-collective) surface as one-off exec failures; retry once
        res = run_bass_kernel_spmd(
            nc, in_maps, core_ids=list(range(NCORES)), trace=trace,
        )
    # out_s[core][b, qc, p, t] -> outT[b, 128*core + p, 512*qc + t]
    stacked = np.stack([r["out_s"] for r in res.results])
    fullT = stacked.transpose(1, 0, 3, 2, 4).reshape(B, D, TQ)
    out = np.ascontiguousarray(
        fullT.astype(np.float32).transpose(0, 2, 1)
    )
    return out, res


def kernel(**inputs):
    out, _ = run(
        inputs["query"], inputs["context"],
        inputs["Wq"], inputs["Wk"], inputs["Wv"], inputs["Wo"],
    )
    return out



# revision 23
# speedup vs baseline: 1.0156x; 1.0156x over previous
"""Trainium2 Bass kernel for chunked (= full, non-causal) cross-attention.

  out = softmax((query Wq^T)(context Wk^T)^T / sqrt(d_head)) (context Wv^T) Wo^T

Shapes: query [2, 2048, 1024], context [2, 4096, 1024], W* [1024, 1024],
16 heads x 64 dims.

Distribution: tensor-parallel over heads.  Core c owns heads {2c, 2c+1}
(128 of the 1024 head dims) for both batches: it holds 128-row slices of
Wq/Wk/Wv and the matching 128-column slice of Wo and computes a
full-shape partial output.

I/O is sharded to minimize host<->device traffic (the axon tunnel is
~40-60MB/s, so replicated activations dominated wall-clock): each core
receives only a 128-row slice of the transposed activations (1/8 of
qT/cT), device-side AllGathers reassemble the full activations in local
DRAM chunk-by-chunk (pipelined under compute), and per-q-chunk
ReduceScatters perform the TP all-reduce of the partial outputs so each
core returns just its 1/8 slice of the final output in bf16.  Host
traffic: ~31MB in + ~8MB out, vs ~330MB for the replicated/host-reduce
variant (8.4s -> ~0.9s warm wall-clock).

On-device layout notes:
  * Activations are fed TRANSPOSED (qT/cT: [B, D, T]) and in bf16 so every
    DMA is contiguous and matmul contraction dims land on partitions.
  * Scores are computed transposed (S^T [k, q]) so softmax's sum over k is
    the AV matmul's contraction; the denominator Z rides along as a fused
    ones-column in the AV stationary operand (M = 64+1).
  * exp runs on the scalar (ACT) engine straight out of PSUM with the
    1/sqrt(64) folded into the activation's free scale; no max-subtraction
    is needed (scores are ~N(0,1); exp stays far below fp32/bf16 limits).
"""

import os
from contextlib import ExitStack

import numpy as np
import ml_dtypes

# Persistent XLA compilation cache: the axon PJRT path re-jits a fresh
# closure every run_bass_kernel_spmd call, which re-runs the BIR->NEFF
# compile (~0.5s).  The persistent cache keys on the HLO fingerprint, so
# warm calls (and future processes) skip it.
try:
    import jax

    jax.config.update("jax_compilation_cache_dir", "/tmp/jax_bass_cache")
    jax.config.update("jax_persistent_cache_min_entry_size_bytes", -1)
    jax.config.update("jax_persistent_cache_min_compile_time_secs", 0)
except Exception:
    pass

import concourse.bass as bass
import concourse.tile as tile
from concourse import bacc, mybir
from concourse.bass_utils import run_bass_kernel_spmd
from concourse.masks import make_identity

B = 2
TQ = 2048
TC = 4096
D = 1024
H = 16
DH = 64
NCORES = 8
E = 128          # head dims owned per core (2 heads x 64)
CT = D // 128    # contraction tiles over d_model
KT = TC // 128   # 128-wide key tiles
QC = TQ // 512   # 512-wide query chunks
KC = TC // 512   # 512-wide key chunks (projection moving dim)
G = 1024         # AllGather chunk granularity (tokens)
CCH = TC // G    # context AG chunks per batch
QCH = TQ // G    # query AG chunks per batch

BF16 = mybir.dt.bfloat16
F32 = mybir.dt.float32

_CACHE = {}
DEBUG = bool(int(os.environ.get("KBG_DEBUG", "0")))


RG = [list(range(NCORES))]


def _build_kernel():
    """Build + compile the per-core Bass module (identical on all cores)."""
    nc = bacc.Bacc(
        "TRN2", target_bir_lowering=False, debug=False, num_devices=NCORES
    )

    # sharded I/O, chunk-major so each AllGather output is contiguous:
    # core c holds rows [128c, 128c+128) of qT/cT per (batch, G-chunk)
    qT_s = nc.dram_tensor("qT_s", [B, QCH, 128, G], BF16, kind="ExternalInput").ap()
    cT_s = nc.dram_tensor("cT_s", [B, CCH, 128, G], BF16, kind="ExternalInput").ap()
    wq = nc.dram_tensor("wq", [D, E], BF16, kind="ExternalInput").ap()
    wk = nc.dram_tensor("wk", [D, E], BF16, kind="ExternalInput").ap()
    wv = nc.dram_tensor("wv", [D, E], BF16, kind="ExternalInput").ap()
    wo = nc.dram_tensor("wo", [64, 2, D], BF16, kind="ExternalInput").ap()
    # reduce-scattered output: core c keeps rows [128c, 128c+128) of
    # out^T per (batch, 512-wide q-chunk)
    out_s = nc.dram_tensor(
        "out_s", [B, QC, 128, 512], BF16, kind="ExternalOutput"
    ).ap()

    # internal DRAM: collective bounce buffers (collectives cannot touch
    # External I/O tensors directly) + gathered activations + TP partials
    qsb = nc.dram_tensor("qsb", [B, QCH, 128, G], BF16, kind="Internal").ap()
    csb = nc.dram_tensor("csb", [B, CCH, 128, G], BF16, kind="Internal").ap()
    qT = nc.dram_tensor("qT_full", [B, QCH, D, G], BF16, kind="Internal").ap()
    cT = nc.dram_tensor("cT_full", [B, CCH, D, G], BF16, kind="Internal").ap()
    out_t = nc.dram_tensor("out_part", [B, QC, D, 512], F32, kind="Internal").ap()
    rs_out = nc.dram_tensor("rs_out", [B, QC, 128, 512], F32, kind="Internal").ap()
    ext = {"out_s": out_s, "rs_out": rs_out}

    dbg = {}
    if DEBUG:
        for name, shape, dt in [
            ("d_qts", [128, TQ], BF16),
            ("d_kts", [128, TC], BF16),
            ("d_vsb", [128, KT, 2, 65], BF16),
            ("d_pt", [128, 2, 512], BF16),
            ("d_rz", [1, 2, 512], F32),
            ("d_rzb", [64, 2, 512], F32),
            ("d_att", [64, 2, 512], BF16),
        ]:
            dbg[name] = nc.dram_tensor(name, shape, dt, kind="ExternalOutput").ap()

    with tile.TileContext(nc) as tc:
        with ExitStack() as ctx:
            # Bounce the input shards to Internal DRAM, then AllGather the
            # full activations chunk by chunk.  Chunk order matches the
            # projection stream's consumption order (c0,q0,c1..c3,q1 per
            # batch), so only the first ~2 chunks are exposed; the rest of
            # the CC stream runs underneath compute.
            nc.sync.dma_start(csb, cT_s)
            nc.sync.dma_start(qsb, qT_s)

            def ag(src, dst):
                nc.gpsimd.collective_compute(
                    "AllGather", mybir.AluOpType.bypass, replica_groups=RG,
                    ins=[src.opt()], outs=[dst.opt()],
                )

            for b in range(B):
                ag(csb[b, 0], cT[b, 0])
                ag(qsb[b, 0], qT[b, 0])
                for cc in range(1, CCH):
                    ag(csb[b, cc], cT[b, cc])
                for qq in range(1, QCH):
                    ag(qsb[b, qq], qT[b, qq])

            _body(ctx, tc, qT, cT, wq, wk, wv, wo, out_t, ext, dbg)

    nc.compile()
    return nc


def _body(ctx, tc, qT, cT, wq, wk, wv, wo, out_t, ext, dbg=None):
    nc = tc.nc
    out_s, rs_out = ext["out_s"], ext["rs_out"]

    const = ctx.enter_context(tc.tile_pool(name="const", bufs=1))
    xq_pool = ctx.enter_context(tc.tile_pool(name="xq", bufs=3))
    xc_pool = ctx.enter_context(tc.tile_pool(name="xc", bufs=4))
    qts_pool = ctx.enter_context(tc.tile_pool(name="qts", bufs=2))
    kts_pool = ctx.enter_context(tc.tile_pool(name="kts", bufs=2))
    vts_pool = ctx.enter_context(tc.tile_pool(name="vts", bufs=1))
    v_pool = ctx.enter_context(tc.tile_pool(name="vsb", bufs=2))
    pt_pool = ctx.enter_context(tc.tile_pool(name="pt", bufs=10))
    avs_pool = ctx.enter_context(tc.tile_pool(name="avs", bufs=2))
    rz_pool = ctx.enter_context(tc.tile_pool(name="rz", bufs=2))
    rzb_pool = ctx.enter_context(tc.tile_pool(name="rzb", bufs=2))
    att_pool = ctx.enter_context(tc.tile_pool(name="att", bufs=2))
    vstage_pool = ctx.enter_context(tc.tile_pool(name="vstage", bufs=4))
    osb_pool = ctx.enter_context(tc.tile_pool(name="osb", bufs=4))
    cast_pool = ctx.enter_context(tc.tile_pool(name="cast", bufs=2))
    dram_pool = ctx.enter_context(tc.tile_pool(name="dram", bufs=2, space="DRAM"))

    sc_psum = ctx.enter_context(tc.tile_pool(name="sc_ps", bufs=2, space="PSUM"))
    av_psum = ctx.enter_context(tc.tile_pool(name="av_ps", bufs=2, space="PSUM"))
    # proj + Wo chains share one double-buffered pool; both are paced
    # one-instruction-at-a-time into the attention stream, so the FIFO
    # slot order can't serialize whole phases against each other.
    misc_psum = ctx.enter_context(tc.tile_pool(name="mi_ps", bufs=2, space="PSUM"))

    # --- constants -----------------------------------------------------
    ident = const.tile([128, 128], BF16)
    make_identity(nc, ident)
    wq_sb = const.tile([128, CT, E], BF16)
    wk_sb = const.tile([128, CT, E], BF16)
    wv_sb = const.tile([128, CT, E], BF16)
    for w_hbm, w_sb in ((wq, wq_sb), (wk, wk_sb), (wv, wv_sb)):
        nc.sync.dma_start(w_sb, w_hbm.rearrange("(ct p) e -> p ct e", p=128))
    wo_sb = const.tile([64, 2, D], BF16)
    nc.sync.dma_start(wo_sb, wo)

    def proj_gen(b, out):
        """Project one batch.  Yields after each PE matmul so the caller
        can pace this work into the attention stream of the previous
        batch (keeps the PE busy but never bursty enough to starve the
        exp pipeline)."""
        # Input chunks live in small ring buffers: slot WAR is at chunk
        # granularity, so the next batch's loads start as soon as this
        # batch's corresponding chains finish (instead of waiting for the
        # whole activation buffer to be released).
        xc_chunks = [None] * KC
        xq_chunks = [None] * QC
        n5 = G // 512  # 512-chunks per AG chunk

        def load_xc(c):
            t = xc_pool.tile([128, CT, 512], BF16, tag="xc")
            src = cT[b, c // n5].rearrange("(ct p) t -> p ct t", p=128)
            nc.sync.dma_start(t, src[:, :, bass.ts(c % n5, 512)])
            xc_chunks[c] = t

        def load_xq(c):
            t = xq_pool.tile([128, CT, 512], BF16, tag="xq")
            src = qT[b, c // n5].rearrange("(ct p) t -> p ct t", p=128)
            nc.sync.dma_start(t, src[:, :, bass.ts(c % n5, 512)])
            xq_chunks[c] = t

        kTs = kts_pool.tile([128, TC], BF16, tag="kts")
        qTs = qts_pool.tile([128, TQ], BF16, tag="qts")
        vTs = vts_pool.tile([128, TC], BF16, tag="vts")
        v_sb = v_pool.tile([128, KT, 2, 65], BF16, tag="vsb")
        nc.vector.memset(v_sb[:, :, :, 64:65], 1.0)
        out.update(kTs=kTs, qTs=qTs, v_sb=v_sb)

        def chain(w_sb, src, dst, c):
            ps = misc_psum.tile([128, 512], F32, tag="mi")
            for ct in range(CT):
                nc.tensor.matmul(
                    ps, w_sb[:, ct, :], src[:, ct, :],
                    start=(ct == 0), stop=(ct == CT - 1),
                )
                yield
            nc.vector.tensor_copy(dst[:, bass.ts(c, 512)], ps)

        def v_transpose(kt):
            # PE transpose: DMA-transpose would force xbar-mode transitions
            # against the copy DMAs sharing the HWDGE queues, which
            # serialize the whole DMA stream (measured as multi-us exp
            # stalls whenever transposes were in flight).
            tp = misc_psum.tile([128, 2, 64], BF16, tag="mi")
            nc.tensor.transpose(tp, vTs[:, bass.ts(kt, 128)], ident)
            nc.vector.tensor_copy(v_sb[:, kt, :, 0:64], tp)
            yield

        # Emission order is a schedule: the PE executes in order, so each
        # chunk must be emitted before the attention iterations that read
        # it.  kt-iteration 4c reads K_c (scores) and V_c (AV), so those
        # chains are emitted V-then-K per chunk; Q_c is only needed when
        # q-chunk c starts, so Q1..Q3 trail at the end.
        load_xc(0)
        load_xq(0)
        load_xc(1)
        yield from chain(wk_sb, xc_chunks[0], kTs, 0)
        yield from chain(wq_sb, xq_chunks[0], qTs, 0)
        load_xc(2)
        yield from chain(wv_sb, xc_chunks[0], vTs, 0)
        for kt in range(4):
            yield from v_transpose(kt)
        for c in range(1, KC):
            if c + 2 < KC:
                load_xc(c + 2)
            yield from chain(wk_sb, xc_chunks[c], kTs, c)
            yield from chain(wv_sb, xc_chunks[c], vTs, c)
            for kt in range(4 * c, 4 * c + 4):
                yield from v_transpose(kt)
        for c in range(1, QC):
            load_xq(c)
            yield from chain(wq_sb, xq_chunks[c], qTs, c)

    def wo_gen(b, qc, att):
        """Output projection for one q-chunk; paced like proj_gen.  Ends
        with the TP reduce-scatter of this q-chunk's partials and the bf16
        downcast of this core's slice, so the collective stream trails the
        compute by ~one q-chunk and only the last one is exposed."""
        for mt in range(D // 128):
            wops = misc_psum.tile([128, 512], F32, tag="mi")
            nc.tensor.matmul(
                wops, wo_sb[:, 0, bass.ts(mt, 128)], att[:, 0, :],
                start=True, stop=False,
            )
            yield
            nc.tensor.matmul(
                wops, wo_sb[:, 1, bass.ts(mt, 128)], att[:, 1, :],
                start=False, stop=True,
            )
            yield
            osb = osb_pool.tile([128, 512], F32, tag="osb")
            nc.vector.tensor_copy(osb, wops)
            nc.sync.dma_start(out_t[b, qc, bass.ts(mt, 128), :], osb)
            yield
        nc.gpsimd.collective_compute(
            "ReduceScatter", mybir.AluOpType.add, replica_groups=RG,
            ins=[out_t[b, qc].opt()], outs=[rs_out[b, qc].opt()],
        )
        yield
        t32 = cast_pool.tile([128, 512], F32, tag="c32")
        nc.sync.dma_start(t32, rs_out[b, qc])
        t16 = cast_pool.tile([128, 512], BF16, tag="c16")
        nc.vector.tensor_copy(t16, t32)
        nc.sync.dma_start(out_s[b, qc], t16)
        yield

    def drive(gens, n):
        done = 0
        while gens and done < n:
            try:
                next(gens[0])
                done += 1
            except StopIteration:
                gens.pop(0)

    proj_pending = []
    wo_pending = []

    # Batch 0: emit loads + chunk-0 projections up front; the rest is
    # paced into the attention stream below (emission position == the
    # PE's execution position, so pacing IS the schedule).
    tensors = [{}, {}]
    proj_pending.append(proj_gen(0, tensors[0]))
    drive(proj_pending, 29)

    for b in range(B):
        kTs, qTs, v_sb = (tensors[b][k] for k in ("kTs", "qTs", "v_sb"))
        if b + 1 < B:
            proj_pending.append(proj_gen(b + 1, tensors[b + 1]))

        for qc in range(QC):
            av0 = av_psum.tile([65, 512], F32, tag="av")
            av1 = av_psum.tile([65, 512], F32, tag="av")
            for kt in range(KT):
                # paced interleave first: producers must be emitted ahead
                # of the iterations that consume them.
                if b == 0 and qc == 0:
                    drive(proj_pending, 5)
                else:
                    drive(proj_pending, 2)
                if kt % 2 == 0:
                    drive(wo_pending, 1)
                sc = sc_psum.tile([128, 2, 512], F32, tag="sc")
                # scores^T [k, q] for the two heads, row-tiled (d=64 each)
                nc.tensor.matmul(
                    sc[:, 0, :], kTs[0:64, bass.ts(kt, 128)],
                    qTs[0:64, bass.ts(qc, 512)], start=True, stop=True,
                )
                nc.tensor.matmul(
                    sc[:, 1, :], kTs[64:128, bass.ts(kt, 128)],
                    qTs[64:128, bass.ts(qc, 512)], start=True, stop=True,
                )
                pt = pt_pool.tile([128, 2, 512], BF16, tag="pt")
                nc.scalar.activation(
                    pt, sc, mybir.ActivationFunctionType.Exp, scale=0.125,
                )
                # AV (+ ones row -> Z at output row 64), accumulate over kt
                nc.tensor.matmul(
                    av0, v_sb[:, kt, 0, :], pt[:, 0, :],
                    start=(kt == 0), stop=(kt == KT - 1),
                )
                nc.tensor.matmul(
                    av1, v_sb[:, kt, 1, :], pt[:, 1, :],
                    start=(kt == 0), stop=(kt == KT - 1),
                )

            # --- stage AV+Z out of PSUM immediately (frees the banks so
            # the next q-chunk starts without draining the pipeline; the
            # slow normalize chain runs on SBUF copies, off the critical
            # path) ----------------------------------------------------
            avs = avs_pool.tile([65, 2, 512], F32, tag="avs")
            nc.vector.tensor_copy(avs[:, 0, :], av0)
            nc.vector.tensor_copy(avs[:, 1, :], av1)

            # --- softmax normalization --------------------------------
            rz = rz_pool.tile([128, 2, 512], F32, tag="rz")
            nc.vector.reciprocal(rz[64:65, :, :], avs[64:65, :, :])
            # Broadcast 1/Z along partitions via a DRAM bounce (engines
            # can't move data across partitions; DMA with a 0-step
            # partition dim from DRAM can).
            rzd = dram_pool.tile([2, 512], F32, tag="rzd")
            nc.sync.dma_start(rzd[0:1, :], rz[64:65, 0, :])
            nc.sync.dma_start(rzd[1:2, :], rz[64:65, 1, :])
            rzb = rzb_pool.tile([64, 2, 512], F32, tag="rzb")
            for j in range(2):
                s = rzd[j : j + 1, :]
                src = bass.AP(
                    tensor=s.tensor, offset=s.offset,
                    ap=[[0, 64]] + [list(d) for d in s.ap[1:]],
                )
                nc.gpsimd.dma_start(rzb[:, j, :], src)
            att = att_pool.tile([64, 2, 512], BF16, tag="att")
            nc.vector.tensor_mul(att[:, 0, :], avs[0:64, 0, :], rzb[:, 0, :])
            nc.vector.tensor_mul(att[:, 1, :], avs[0:64, 1, :], rzb[:, 1, :])

            wo_pending.append(wo_gen(b, qc, att))

    # drain whatever interleaved work remains
    drive(proj_pending, 1 << 30)
    drive(wo_pending, 1 << 30)


def _prep_inputs(query, context, Wq, Wk, Wv, Wo):
    """Host-side sharding: bf16 casts, transposes, per-core weight slices."""
    bf16 = ml_dtypes.bfloat16

    def t_bf16(x):
        """Cast then blocked-transpose [B,T,D] -> [B,D,T]; the straight
        strided astype reads one cache line per 2-byte element."""
        b, t, d = x.shape
        x16 = x.astype(bf16)
        out = np.empty((b, d, t), bf16)
        xt = x16.transpose(0, 2, 1)
        for t0 in range(0, t, 256):
            out[:, :, t0 : t0 + 256] = xt[:, :, t0 : t0 + 256]
        return out

    qT = t_bf16(query)
    cT = t_bf16(context)
    in_maps = []
    for c in range(NCORES):
        sl = slice(E * c, E * (c + 1))
        rs = slice(128 * c, 128 * (c + 1))
        wo_slice = np.ascontiguousarray(Wo[:, sl].T)          # [128 e, 1024 m]
        wo_dev = np.ascontiguousarray(
            wo_slice.reshape(2, 64, D).transpose(1, 0, 2)      # [64, 2, 1024]
        ).astype(bf16)
        in_maps.append({
            "qT_s": np.ascontiguousarray(
                qT[:, rs, :].reshape(B, 128, QCH, G).transpose(0, 2, 1, 3)
            ),
            "cT_s": np.ascontiguousarray(
                cT[:, rs, :].reshape(B, 128, CCH, G).transpose(0, 2, 1, 3)
            ),
            "wq": np.ascontiguousarray(Wq[sl, :].T).astype(bf16),
            "wk": np.ascontiguousarray(Wk[sl, :].T).astype(bf16),
            "wv": np.ascontiguousarray(Wv[sl, :].T).astype(bf16),
            "wo": wo_dev,
        })
    return in_maps


def run(query, context, Wq, Wk, Wv, Wo, trace=False):
    """Run on 8 cores; returns (full output [B, TQ, D] fp32, BassKernelResults)."""
    if "nc" not in _CACHE:
        _CACHE["nc"] = _build_kernel()
    nc = _CACHE["nc"]
    in_maps = _prep_inputs(query, context, Wq, Wk, Wv, Wo)
    try:
        res = run_bass_kernel_spmd(
            nc, in_maps, core_ids=list(range(NCORES)), trace=trace,
        )
    except Exception:
        # transient device/tunnel states (e.g. a prior process killed
        # mid-collective) surface as one-off exec failures; retry once
        res = run_bass_kernel_spmd(
            nc, in_maps, core_ids=list(range(NCORES)), trace=trace,
        )
    # out_s[core][b, qc, p, t] -> outT[b, 128*core + p, 512*qc + t]
    stacked = np.stack([r["out_s"] for r in res.results])
    fullT = stacked.transpose(1, 0, 3, 2, 4).reshape(B, D, TQ)
    out = np.ascontiguousarray(
        fullT.astype(np.float32).transpose(0, 2, 1)
    )
    return out, res


def kernel(**inputs):
    out, _ = run(
        inputs["query"], inputs["context"],
        inputs["Wq"], inputs["Wk"], inputs["Wv"], inputs["Wo"],
    )
    return out

